# revision 25
# baseline (speedup 1.0000x reference)
"""GCN layer kernel for Trainium2 (8 NeuronCores, SPMD).

out = relu( D^{-1/2} (A+I) D^{-1/2} x W^T + b )

Math restructure (v2 — aggregate-then-project):
    xs[j] = dinv[j] * x[j]                      (host-baked, bf16 table)
    agg[i] = dinv[i] * ( sum_{(i,j) in E} xs[j] + xs[i] )
    out[i] = relu( agg[i] @ W^T + b )

Device plan per core (core c owns src-node rows [c*6250, (c+1)*6250)):
  For each 128-src-node chunk: dma_gather xs[dst] rows for the chunk's
  (host-bucketed, src-sorted, self-loop-free) edges, build one-hot
  selection matrices S on the DVE (slot-id vs iota compare) and
  segment-reduce with PE matmuls accumulating in PSUM [slot, fi]; the
  chunk's own xs tile (contiguous, HWDGE) adds the self-loops via an
  identity matmul.  Scale by dinv[src] during the PSUM->SBUF copy,
  transpose on the PE, project with W^T halves (+ bias via a one-hot
  matmul), relu, and store the output rows (grouped DMA).

Host does only sharding/layout work: edge bucketing by (core, chunk,
dst-half), int16 gather-index packing, degree counting, scaling/casts.
"""

import sys

for _p in ("/opt/trn_rl_repo",):
    if _p not in sys.path:
        sys.path.insert(0, _p)

from contextlib import ExitStack

import ml_dtypes
import numpy as np

import concourse.bass as bass
import concourse.mybir as mybir
import concourse.tile as tile
from concourse import bacc
from concourse.bass_utils import run_bass_kernel_spmd

BF16 = ml_dtypes.bfloat16

N_NODES = 50000
N_EDGES = 800000
F = 256  # in_size == out_size == 256
N_CORES = 8
NPC = N_NODES // N_CORES  # 6250 nodes per core
SPLIT = 32768  # int16 index limit for dma_gather
NT_PAD = 50048  # 391 * 128, padded node count for the xs table
CHUNKS = (NPC + 127) // 128  # 49 chunks of <=128 src nodes per core
OUT_GRP = 8  # output chunks per DRAM write


def _pack_idx(vals, blocks):
    """int16 gather index layout: position i -> [i % 16, i // 16],
    replicated to 128 partitions."""
    n = blocks * 128
    a = np.zeros(n, dtype=np.int16)
    a[: len(vals)] = vals
    cols = a.reshape(n // 16, 16).T  # [16, n/16]
    return np.tile(cols, (8, 1))  # [128, n/16]


def _pack_slots(vals, blocks, pad_val=200.0):
    """slot layout: position i -> [i % 128, i // 128]."""
    n = blocks * 128
    a = np.full(n, pad_val, dtype=np.float32)
    a[: len(vals)] = vals
    return a.reshape(blocks, 128).T.astype(BF16)  # [128, blocks]


def _build_program(lo_blk, hi_blk, single_packet=False):
    """Build the (core-uniform) Bass program. lo_blk/hi_blk: per-chunk
    gather block counts (lists of CHUNKS ints)."""
    # 4 SWDGE queues: a dma_gather on queue q runs its descriptor
    # generation on Q7 core pair (2q, 2q+1), so round-robining the
    # gathers over queues 0-3 runs up to 4 generations concurrently.
    nc = bacc.Bacc(
        None, target_bir_lowering=False, debug=False, num_swdge_queues=4
    )
    dt = mybir.dt

    sum_lo = int(sum(lo_blk))
    sum_hi = int(sum(hi_blk))
    sum_nb = sum_lo + sum_hi

    xs = nc.dram_tensor("xs", [NT_PAD, F], dt.bfloat16, kind="ExternalInput")
    wT = nc.dram_tensor("wt", [2, 128, F], dt.bfloat16, kind="ExternalInput")
    biasw = nc.dram_tensor("biasw", [128, F], dt.bfloat16, kind="ExternalInput")
    iota = nc.dram_tensor("iota", [128, 128], dt.bfloat16, kind="ExternalInput")
    ident = nc.dram_tensor("ident", [128, 128], dt.bfloat16, kind="ExternalInput")
    onehot0 = nc.dram_tensor("onehot0", [128, 128], dt.bfloat16, kind="ExternalInput")
    dinv_chk = nc.dram_tensor("dinv_chk", [128, CHUNKS], dt.float32, kind="ExternalInput")
    idx_lo = nc.dram_tensor("idx_lo", [128, 8 * sum_lo], dt.int16, kind="ExternalInput")
    idx_hi = nc.dram_tensor("idx_hi", [128, 8 * sum_hi], dt.int16, kind="ExternalInput")
    slots = nc.dram_tensor("slots", [128, sum_nb], dt.bfloat16, kind="ExternalInput")
    xself = nc.dram_tensor("xself", [CHUNKS * 128, F], dt.bfloat16, kind="ExternalInput")
    out = nc.dram_tensor("out", [NPC, F], dt.float32, kind="ExternalOutput")

    with tile.TileContext(nc) as tc, ExitStack() as top:
        cpool = top.enter_context(tc.tile_pool(name="const", bufs=1))
        # gather indices load first (gathers are the critical path)
        ilo_s = cpool.tile([128, 8 * sum_lo], dt.int16)
        nc.sync.dma_start(out=ilo_s[:], in_=idx_lo[:])
        ihi_s = cpool.tile([128, 8 * sum_hi], dt.int16)
        nc.sync.dma_start(out=ihi_s[:], in_=idx_hi[:])
        slt_s = cpool.tile([128, sum_nb], dt.bfloat16)
        nc.sync.dma_start(out=slt_s[:], in_=slots[:])
        wt_s = cpool.tile([128, 2, F], dt.bfloat16)
        nc.sync.dma_start(out=wt_s[:, 0, :], in_=wT[0])
        nc.sync.dma_start(out=wt_s[:, 1, :], in_=wT[1])
        bw_s = cpool.tile([128, F], dt.bfloat16)
        nc.sync.dma_start(out=bw_s[:], in_=biasw[:])
        iota_s = cpool.tile([128, 128], dt.bfloat16)
        nc.sync.dma_start(out=iota_s[:], in_=iota[:])
        id_s = cpool.tile([128, 128], dt.bfloat16)
        nc.sync.dma_start(out=id_s[:], in_=ident[:])
        oh_s = cpool.tile([128, 128], dt.bfloat16)
        nc.sync.dma_start(out=oh_s[:], in_=onehot0[:])
        dvc_s = cpool.tile([128, CHUNKS], dt.float32)
        nc.sync.dma_start(out=dvc_s[:], in_=dinv_chk[:])

        with ExitStack() as p2:
            gpool = p2.enter_context(tc.tile_pool(name="gat", bufs=4))
            xpool = p2.enter_context(tc.tile_pool(name="xself", bufs=3))
            spool = p2.enter_context(tc.tile_pool(name="sel", bufs=3))
            apool = p2.enter_context(tc.tile_pool(name="aggs", bufs=3))
            tpool = p2.enter_context(tc.tile_pool(name="aggt", bufs=3))
            opool = p2.enter_context(tc.tile_pool(name="ostg", bufs=2))
            psA = p2.enter_context(tc.tile_pool(name="psA", bufs=3, space="PSUM"))
            psT = p2.enter_context(tc.tile_pool(name="psT", bufs=2, space="PSUM"))
            psO = p2.enter_context(tc.tile_pool(name="psO", bufs=2, space="PSUM"))

            lo_off = 0
            hi_off = 0
            nb_off = 0
            ob = None
            ob_base = 0
            og = 0
            qn = 0
            GC = 2  # chunks per gather call (amortizes per-call fixed cost)
            for g0 in range(0, CHUNKS, GC):
                ks = list(range(g0, min(g0 + GC, CHUNKS)))
                LOs = [int(lo_blk[k]) for k in ks]
                HIs = [int(hi_blk[k]) for k in ks]
                LOg, HIg = sum(LOs), sum(HIs)
                Glo = gpool.tile([128, LOg, F], dt.bfloat16, tag="Glo")
                nc.gpsimd.dma_gather(
                    Glo[:],
                    xs[0:SPLIT, :],
                    ilo_s[:, 8 * lo_off : 8 * (lo_off + LOg)],
                    128 * LOg,
                    128 * LOg,
                    F,
                    single_packet=False,
                    queue_num=qn,
                )
                qn = (qn + 1) % 4
                Ghi = gpool.tile([128, HIg, F], dt.bfloat16, tag="Ghi")
                nc.gpsimd.dma_gather(
                    Ghi[:],
                    xs[SPLIT:NT_PAD, :],
                    ihi_s[:, 8 * hi_off : 8 * (hi_off + HIg)],
                    128 * HIg,
                    128 * HIg,
                    F,
                    single_packet=False,
                    queue_num=qn,
                )
                qn = (qn + 1) % 4
                lo_b = 0
                hi_b = 0
                for i, k in enumerate(ks):
                    LO, HI = LOs[i], HIs[i]
                    NB = LO + HI
                    if ob is None:
                        og = min(OUT_GRP, CHUNKS - k)
                        ob = opool.tile([128, og, F], dt.float32, tag="ob")
                        ob_base = k
                    # chunk's own xs rows (contiguous) for the self-loops
                    xsf = xpool.tile([128, F], dt.bfloat16, tag="xsf")
                    nc.sync.dma_start(
                        out=xsf[:], in_=xself[k * 128 : (k + 1) * 128, :]
                    )
                    ps = psA.tile([128, F], dt.float32)
                    # one DVE op builds all NB one-hot S blocks for this chunk
                    S = spool.tile([128, NB, 128], dt.bfloat16, tag="S")
                    nc.vector.tensor_tensor(
                        out=S[:],
                        in0=slt_s[:, nb_off : nb_off + NB].to_broadcast(
                            [128, NB, 128]
                        ),
                        in1=iota_s[:, None, :].to_broadcast([128, NB, 128]),
                        op=mybir.AluOpType.is_equal,
                    )
                    for b in range(LO):
                        nc.tensor.matmul(
                            out=ps[:],
                            lhsT=S[:, b, :],
                            rhs=Glo[:, lo_b + b, :],
                            start=(b == 0),
                            stop=False,
                        )
                    for b in range(HI):
                        nc.tensor.matmul(
                            out=ps[:],
                            lhsT=S[:, LO + b, :],
                            rhs=Ghi[:, hi_b + b, :],
                            start=False,
                            stop=False,
                        )
                    # self-loop rows via identity matmul
                    nc.tensor.matmul(
                        out=ps[:], lhsT=id_s[:], rhs=xsf[:],
                        start=False, stop=True,
                    )
                    # agg_sb = dinv[src-slot] * psA   (bf16)
                    agg = apool.tile([128, F], dt.bfloat16, tag="agg")
                    nc.vector.tensor_scalar(
                        out=agg[:],
                        in0=ps[:],
                        scalar1=dvc_s[:, k : k + 1],
                        scalar2=None,
                        op0=mybir.AluOpType.mult,
                    )
                    # transpose agg on the PE (transpose-mode PSUM out
                    # matches the lhsT dtype, bf16 — no cast on the copy)
                    pt = psT.tile([128, 2, 128], dt.bfloat16)
                    nc.tensor.transpose(pt[:, 0, :], agg[:, 0:128], id_s[:])
                    nc.tensor.transpose(pt[:, 1, :], agg[:, 128:256], id_s[:])
                    at = tpool.tile([128, 2, 128], dt.bfloat16, tag="at")
                    nc.scalar.activation(
                        out=at[:], in_=pt[:],
                        func=mybir.ActivationFunctionType.Copy,
                    )
                    # projection + bias
                    po = psO.tile([128, F], dt.float32)
                    nc.tensor.matmul(
                        out=po[:], lhsT=at[:, 0, :], rhs=wt_s[:, 0, :],
                        start=True, stop=False,
                    )
                    nc.tensor.matmul(
                        out=po[:], lhsT=at[:, 1, :], rhs=wt_s[:, 1, :],
                        start=False, stop=False,
                    )
                    nc.tensor.matmul(
                        out=po[:], lhsT=oh_s[:], rhs=bw_s[:],
                        start=False, stop=True,
                    )
                    nc.scalar.activation(
                        out=ob[:, k - ob_base, :],
                        in_=po[:],
                        func=mybir.ActivationFunctionType.Relu,
                    )
                    if k - ob_base + 1 == og:
                        r0 = ob_base * 128
                        rw = og * 128
                        if r0 + rw <= NPC:
                            dst = out[r0 : r0 + rw, :].rearrange(
                                "(t p) f -> p t f", p=128
                            )
                            nc.sync.dma_start(out=dst, in_=ob[:])
                        else:
                            # tail group: full chunks + one partial (106)
                            full = (NPC - r0) // 128
                            if full:
                                dst = out[r0 : r0 + full * 128, :].rearrange(
                                    "(t p) f -> p t f", p=128
                                )
                                nc.sync.dma_start(out=dst, in_=ob[:, :full, :])
                            rem = NPC - r0 - full * 128
                            if rem:
                                nc.sync.dma_start(
                                    out=out[r0 + full * 128 : NPC, :],
                                    in_=ob[:rem, full, :],
                                )
                        ob = None
                    lo_b += LO
                    hi_b += HI
                    nb_off += NB
                lo_off += LOg
                hi_off += HIg

    nc.compile()
    return nc


def _prep(x, edge_index, W, b):
    """Host-side sharding/layout. Returns (lo_blk, hi_blk, common, per_core)."""
    src = np.asarray(edge_index[0], dtype=np.int64)
    dst = np.asarray(edge_index[1], dtype=np.int64)
    deg = np.bincount(src, minlength=N_NODES).astype(np.float32)
    dinv = deg**-0.5

    core = src // NPC
    src_local = src - core * NPC
    chunk = src_local >> 7
    slot = src_local & 127
    is_hi = (dst >= SPLIT).astype(np.int64)
    key = (core * CHUNKS + chunk) * 2 + is_hi
    order = np.argsort(key, kind="stable")
    key_s = key[order]
    dst_s = dst[order]
    slot_s = slot[order]

    nseg = N_CORES * CHUNKS * 2
    counts = np.bincount(key_s, minlength=nseg).reshape(N_CORES, CHUNKS, 2)
    seg_end = np.cumsum(counts.reshape(-1))
    seg_start = seg_end - counts.reshape(-1)

    lo_max = counts[:, :, 0].max(axis=0)  # [CHUNKS]
    hi_max = counts[:, :, 1].max(axis=0)
    lo_blk = np.maximum(1, (lo_max + 127) // 128).astype(np.int64)
    hi_blk = np.maximum(1, (hi_max + 127) // 128).astype(np.int64)

    # common (replicated) tensors
    xs_t = np.zeros((NT_PAD, F), dtype=BF16)
    xs_t[:N_NODES] = (
        np.asarray(x, dtype=np.float32) * dinv[:, None]
    ).astype(BF16)
    wT = np.ascontiguousarray(np.asarray(W, dtype=np.float32).T).astype(BF16)
    wt_in = np.stack([wT[:128], wT[128:]])  # [2,128,F]
    biasw = np.zeros((128, F), dtype=BF16)
    biasw[0, :] = np.asarray(b, dtype=np.float32).astype(BF16)
    iota_t = np.tile(np.arange(128, dtype=np.float32)[None, :], (128, 1)).astype(BF16)
    ident_t = np.eye(128, dtype=np.float32).astype(BF16)
    onehot0 = np.zeros((128, 128), dtype=BF16)
    onehot0[0, :] = 1.0
    common = dict(
        xs=xs_t, wt=wt_in, biasw=biasw, iota=iota_t, ident=ident_t,
        onehot0=onehot0,
    )

    dinv_pad = np.ones(NT_PAD, dtype=np.float32)
    dinv_pad[:N_NODES] = dinv

    sum_lo = int(lo_blk.sum())
    sum_hi = int(hi_blk.sum())
    per_core = []
    for c in range(N_CORES):
        ilo = np.zeros((128, 8 * sum_lo), dtype=np.int16)
        ihi = np.zeros((128, 8 * sum_hi), dtype=np.int16)
        slt = np.zeros((128, sum_lo + sum_hi), dtype=BF16)
        lo_off = hi_off = nb_off = 0
        for k in range(CHUNKS):
            LO, HI = int(lo_blk[k]), int(hi_blk[k])
            s = (c * CHUNKS + k) * 2
            a0, a1 = seg_start[s], seg_end[s]
            b0, b1 = seg_start[s + 1], seg_end[s + 1]
            ilo[:, 8 * lo_off : 8 * (lo_off + LO)] = _pack_idx(dst_s[a0:a1], LO)
            ihi[:, 8 * hi_off : 8 * (hi_off + HI)] = _pack_idx(
                dst_s[b0:b1] - SPLIT, HI
            )
            slt[:, nb_off : nb_off + LO] = _pack_slots(slot_s[a0:a1], LO)
            slt[:, nb_off + LO : nb_off + LO + HI] = _pack_slots(slot_s[b0:b1], HI)
            lo_off += LO
            hi_off += HI
            nb_off += LO + HI
        nchk = np.arange(128)[:, None] + 128 * np.arange(CHUNKS)[None, :] + c * NPC
        dvc = np.where(
            nchk - c * NPC < NPC, dinv_pad[np.minimum(nchk, N_NODES - 1)], 1.0
        ).astype(np.float32)
        per_core.append(
            dict(
                idx_lo=ilo,
                idx_hi=ihi,
                slots=slt,
                dinv_chk=np.ascontiguousarray(dvc),
                xself=np.ascontiguousarray(
                    xs_t[c * NPC : c * NPC + CHUNKS * 128]
                ),
            )
        )
    return lo_blk, hi_blk, common, per_core


def _install_ntff_hook():
    """The agent image's antenv lacks axon_hooks; recreate it so
    run_bass_kernel_spmd(trace=True) can profile via the axon .so."""
    import types

    if "antenv.axon_hooks" in sys.modules:
        return
    mod = types.ModuleType("antenv.axon_hooks")
    state = {}
    mod.set_axon_ntff_profile_hook = lambda h: state.__setitem__("h", h)
    mod.get_axon_ntff_profile_hook = lambda: state.get("h")
    sys.modules["antenv.axon_hooks"] = mod
    try:
        import antenv

        antenv.axon_hooks = mod
    except Exception:
        pass
    try:
        if "/root/.axon_site" not in sys.path:
            sys.path.insert(0, "/root/.axon_site")
        from trn_agent_boot.trn_boot import _ntff_profile_via_ctypes

        mod.set_axon_ntff_profile_hook(
            _ntff_profile_via_ctypes("/opt/axon/libaxon_pjrt.so")
        )
    except Exception:
        pass


_CACHE = {}


def kernel(x, edge_index, W, b, trace=False):
    if trace:
        _install_ntff_hook()
    lo_blk, hi_blk, common, per_core = _prep(x, edge_index, W, b)
    key = (tuple(lo_blk), tuple(hi_blk))
    if key not in _CACHE:
        _CACHE[key] = _build_program(lo_blk, hi_blk)
    nc = _CACHE[key]

    in_maps = []
    for c in range(N_CORES):
        m = dict(common)
        m.update(per_core[c])
        in_maps.append(m)

    res = run_bass_kernel_spmd(
        nc, in_maps, core_ids=list(range(N_CORES)), trace=trace
    )
    out = np.concatenate([r["out"] for r in res.results], axis=0)
    if trace:
        kernel.last_exec_ns = res.exec_time_ns
        kernel.last_profile = res.profile_json
    return out.astype(np.float32)


# revision 27
# speedup vs baseline: 1.1298x; 1.1298x over previous
"""GCN layer kernel for Trainium2 (8 NeuronCores, SPMD).

out = relu( D^{-1/2} (A+I) D^{-1/2} x W^T + b )

Math restructure (v2 — aggregate-then-project):
    xs[j] = dinv[j] * x[j]                      (host-baked, bf16 table)
    agg[i] = dinv[i] * ( sum_{(i,j) in E} xs[j] + xs[i] )
    out[i] = relu( agg[i] @ W^T + b )

Device plan per core (core c owns src-node rows [c*6250, (c+1)*6250)):
  For each 128-src-node chunk: dma_gather xs[dst] rows for the chunk's
  (host-bucketed, src-sorted, self-loop-free) edges, build one-hot
  selection matrices S on the DVE (slot-id vs iota compare) and
  segment-reduce with PE matmuls accumulating in PSUM [slot, fi]; the
  chunk's own xs tile (contiguous, HWDGE) adds the self-loops via an
  identity matmul.  Scale by dinv[src] during the PSUM->SBUF copy,
  transpose on the PE, project with W^T halves (+ bias via a one-hot
  matmul), relu, and store the output rows (grouped DMA).

Host does only sharding/layout work: edge bucketing by (core, chunk,
dst-half), int16 gather-index packing, degree counting, scaling/casts.
"""

import sys

for _p in ("/opt/trn_rl_repo",):
    if _p not in sys.path:
        sys.path.insert(0, _p)

from contextlib import ExitStack

import ml_dtypes
import numpy as np

import concourse.bass as bass
import concourse.mybir as mybir
import concourse.tile as tile
from concourse import bacc
from concourse.bass_utils import run_bass_kernel_spmd

BF16 = ml_dtypes.bfloat16

N_NODES = 50000
N_EDGES = 800000
F = 256  # in_size == out_size == 256
N_CORES = 8
NPC = N_NODES // N_CORES  # 6250 nodes per core
SPLIT = 32768  # int16 index limit for dma_gather
NT_PAD = 50048  # 391 * 128, padded node count for the xs table
CHUNKS = (NPC + 127) // 128  # 49 chunks of <=128 src nodes per core
OUT_GRP = 8  # output chunks per DRAM write


def _pack_idx(vals, blocks):
    """int16 gather index layout: position i -> [i % 16, i // 16],
    replicated to 128 partitions."""
    n = blocks * 128
    a = np.zeros(n, dtype=np.int16)
    a[: len(vals)] = vals
    cols = a.reshape(n // 16, 16).T  # [16, n/16]
    return np.tile(cols, (8, 1))  # [128, n/16]


def _pack_slots(vals, blocks, pad_val=200.0):
    """slot layout: position i -> [i % 128, i // 128]."""
    n = blocks * 128
    a = np.full(n, pad_val, dtype=np.float32)
    a[: len(vals)] = vals
    return a.reshape(blocks, 128).T.astype(BF16)  # [128, blocks]


def _build_program(lo_blk, hi_blk, single_packet=False):
    """Build the (core-uniform) Bass program. lo_blk/hi_blk: per-chunk
    gather block counts (lists of CHUNKS ints)."""
    # 4 SWDGE queues: a dma_gather on queue q runs its descriptor
    # generation on Q7 core pair (2q, 2q+1), so round-robining the
    # gathers over queues 0-3 runs up to 4 generations concurrently.
    nc = bacc.Bacc(
        None, target_bir_lowering=False, debug=False, num_swdge_queues=4
    )
    dt = mybir.dt

    sum_lo = int(sum(lo_blk))
    sum_hi = int(sum(hi_blk))
    sum_nb = sum_lo + sum_hi

    xs = nc.dram_tensor("xs", [NT_PAD, F], dt.bfloat16, kind="ExternalInput")
    wT = nc.dram_tensor("wt", [2, 128, F], dt.bfloat16, kind="ExternalInput")
    biasw = nc.dram_tensor("biasw", [128, F], dt.bfloat16, kind="ExternalInput")
    iota = nc.dram_tensor("iota", [128, 128], dt.bfloat16, kind="ExternalInput")
    ident = nc.dram_tensor("ident", [128, 128], dt.bfloat16, kind="ExternalInput")
    onehot0 = nc.dram_tensor("onehot0", [128, 128], dt.bfloat16, kind="ExternalInput")
    dinv_chk = nc.dram_tensor("dinv_chk", [128, CHUNKS], dt.float32, kind="ExternalInput")
    idx_lo = nc.dram_tensor("idx_lo", [128, 8 * sum_lo], dt.int16, kind="ExternalInput")
    idx_hi = nc.dram_tensor("idx_hi", [128, 8 * sum_hi], dt.int16, kind="ExternalInput")
    slots = nc.dram_tensor("slots", [128, sum_nb], dt.bfloat16, kind="ExternalInput")
    xself = nc.dram_tensor("xself", [CHUNKS * 128, F], dt.bfloat16, kind="ExternalInput")
    out = nc.dram_tensor("out", [NPC, F], dt.float32, kind="ExternalOutput")

    with tile.TileContext(nc) as tc, ExitStack() as top:
        cpool = top.enter_context(tc.tile_pool(name="const", bufs=1))
        # gather indices load first (gathers are the critical path)
        ilo_s = cpool.tile([128, 8 * sum_lo], dt.int16)
        nc.sync.dma_start(out=ilo_s[:], in_=idx_lo[:])
        ihi_s = cpool.tile([128, 8 * sum_hi], dt.int16)
        nc.sync.dma_start(out=ihi_s[:], in_=idx_hi[:])
        slt_s = cpool.tile([128, sum_nb], dt.bfloat16)
        nc.sync.dma_start(out=slt_s[:], in_=slots[:])
        wt_s = cpool.tile([128, 2, F], dt.bfloat16)
        nc.sync.dma_start(out=wt_s[:, 0, :], in_=wT[0])
        nc.sync.dma_start(out=wt_s[:, 1, :], in_=wT[1])
        bw_s = cpool.tile([128, F], dt.bfloat16)
        nc.sync.dma_start(out=bw_s[:], in_=biasw[:])
        iota_s = cpool.tile([128, 128], dt.bfloat16)
        nc.sync.dma_start(out=iota_s[:], in_=iota[:])
        id_s = cpool.tile([128, 128], dt.bfloat16)
        nc.sync.dma_start(out=id_s[:], in_=ident[:])
        oh_s = cpool.tile([128, 128], dt.bfloat16)
        nc.sync.dma_start(out=oh_s[:], in_=onehot0[:])
        dvc_s = cpool.tile([128, CHUNKS], dt.float32)
        nc.sync.dma_start(out=dvc_s[:], in_=dinv_chk[:])

        with ExitStack() as p2:
            gpool = p2.enter_context(tc.tile_pool(name="gat", bufs=7))
            xpool = p2.enter_context(tc.tile_pool(name="xself", bufs=3))
            spool = p2.enter_context(tc.tile_pool(name="sel", bufs=3))
            apool = p2.enter_context(tc.tile_pool(name="aggs", bufs=3))
            tpool = p2.enter_context(tc.tile_pool(name="aggt", bufs=3))
            opool = p2.enter_context(tc.tile_pool(name="ostg", bufs=2))
            psA = p2.enter_context(tc.tile_pool(name="psA", bufs=3, space="PSUM"))
            psT = p2.enter_context(tc.tile_pool(name="psT", bufs=2, space="PSUM"))
            psO = p2.enter_context(tc.tile_pool(name="psO", bufs=2, space="PSUM"))

            state = dict(ob=None, ob_base=0, og=0)

            def back_half(k, agg):
                # transpose agg on the PE (transpose-mode PSUM out matches
                # the lhsT dtype, bf16), project with W^T halves + bias,
                # relu into the staging group, flush the group when full.
                if state["ob"] is None:
                    state["og"] = min(OUT_GRP, CHUNKS - k)
                    ob_t = opool.tile(
                        [128, state["og"], F], dt.float32, tag="ob"
                    )
                    state["ob"] = ob_t
                    state["ob_base"] = k
                ob = state["ob"]
                pt = psT.tile([128, 2, 128], dt.bfloat16)
                nc.tensor.transpose(pt[:, 0, :], agg[:, 0:128], id_s[:])
                nc.tensor.transpose(pt[:, 1, :], agg[:, 128:256], id_s[:])
                at = tpool.tile([128, 2, 128], dt.bfloat16, tag="at")
                nc.scalar.activation(
                    out=at[:], in_=pt[:],
                    func=mybir.ActivationFunctionType.Copy,
                )
                po = psO.tile([128, F], dt.float32)
                nc.tensor.matmul(
                    out=po[:], lhsT=at[:, 0, :], rhs=wt_s[:, 0, :],
                    start=True, stop=False,
                )
                nc.tensor.matmul(
                    out=po[:], lhsT=at[:, 1, :], rhs=wt_s[:, 1, :],
                    start=False, stop=False,
                )
                nc.tensor.matmul(
                    out=po[:], lhsT=oh_s[:], rhs=bw_s[:],
                    start=False, stop=True,
                )
                nc.scalar.activation(
                    out=ob[:, k - state["ob_base"], :],
                    in_=po[:],
                    func=mybir.ActivationFunctionType.Relu,
                )
                if k - state["ob_base"] + 1 == state["og"]:
                    r0 = state["ob_base"] * 128
                    rw = state["og"] * 128
                    if r0 + rw <= NPC:
                        dst = out[r0 : r0 + rw, :].rearrange(
                            "(t p) f -> p t f", p=128
                        )
                        nc.sync.dma_start(out=dst, in_=ob[:])
                    else:
                        # tail group: full chunks + one partial (106 rows)
                        full = (NPC - r0) // 128
                        if full:
                            dst = out[r0 : r0 + full * 128, :].rearrange(
                                "(t p) f -> p t f", p=128
                            )
                            nc.sync.dma_start(out=dst, in_=ob[:, :full, :])
                        rem = NPC - r0 - full * 128
                        if rem:
                            nc.sync.dma_start(
                                out=out[r0 + full * 128 : NPC, :],
                                in_=ob[:rem, full, :],
                            )
                    state["ob"] = None

            lo_off = 0
            hi_off = 0
            nb_off = 0
            qn = 0
            prev = None  # (k, agg) pending back-half, one chunk behind
            for k in range(CHUNKS):
                LO, HI = int(lo_blk[k]), int(hi_blk[k])
                NB = LO + HI
                G = gpool.tile([128, NB, F], dt.bfloat16, tag="G")
                if LO:
                    nc.gpsimd.dma_gather(
                        G[:, 0:LO, :],
                        xs[0:SPLIT, :],
                        ilo_s[:, 8 * lo_off : 8 * (lo_off + LO)],
                        128 * LO,
                        128 * LO,
                        F,
                        single_packet=False,
                        queue_num=qn,
                    )
                    qn = (qn + 1) % 4
                if HI:
                    nc.gpsimd.dma_gather(
                        G[:, LO:NB, :],
                        xs[SPLIT:NT_PAD, :],
                        ihi_s[:, 8 * hi_off : 8 * (hi_off + HI)],
                        128 * HI,
                        128 * HI,
                        F,
                        single_packet=False,
                        queue_num=qn,
                    )
                    qn = (qn + 1) % 4
                # chunk's own xs rows (contiguous) for the self-loops
                xsf = xpool.tile([128, F], dt.bfloat16, tag="xsf")
                nc.sync.dma_start(
                    out=xsf[:], in_=xself[k * 128 : (k + 1) * 128, :]
                )
                ps = psA.tile([128, F], dt.float32)
                # one DVE op builds all NB one-hot S blocks for this chunk;
                # the DVE runs ONLY S-builds so chunk k+1's S is never
                # stuck behind a PE-dependent op in the DVE FIFO.
                S = spool.tile([128, NB, 128], dt.bfloat16, tag="S")
                nc.vector.tensor_tensor(
                    out=S[:],
                    in0=slt_s[:, nb_off : nb_off + NB].to_broadcast(
                        [128, NB, 128]
                    ),
                    in1=iota_s[:, None, :].to_broadcast([128, NB, 128]),
                    op=mybir.AluOpType.is_equal,
                )
                for b in range(NB):
                    nc.tensor.matmul(
                        out=ps[:],
                        lhsT=S[:, b, :],
                        rhs=G[:, b, :],
                        start=(b == 0),
                        stop=False,
                    )
                # self-loop rows via identity matmul
                nc.tensor.matmul(
                    out=ps[:], lhsT=id_s[:], rhs=xsf[:], start=False, stop=True
                )
                # agg = dinv[src-slot] * psA (bf16), on the Scalar engine
                agg = apool.tile([128, F], dt.bfloat16, tag="agg")
                nc.scalar.activation(
                    out=agg[:], in_=ps[:],
                    func=mybir.ActivationFunctionType.Copy,
                    scale=dvc_s[:, k : k + 1],
                )
                # epilogue of the PREVIOUS chunk (keeps the PE FIFO free of
                # ops that would stall the next chunk's segment matmuls)
                if prev is not None:
                    back_half(*prev)
                prev = (k, agg)
                lo_off += LO
                hi_off += HI
                nb_off += NB
            back_half(*prev)

    nc.compile()
    return nc


def _prep(x, edge_index, W, b):
    """Host-side sharding/layout. Returns (lo_blk, hi_blk, common, per_core)."""
    src = np.asarray(edge_index[0], dtype=np.int64)
    dst = np.asarray(edge_index[1], dtype=np.int64)
    deg = np.bincount(src, minlength=N_NODES).astype(np.float32)
    dinv = deg**-0.5

    core = src // NPC
    src_local = src - core * NPC
    chunk = src_local >> 7
    slot = src_local & 127
    is_hi = (dst >= SPLIT).astype(np.int64)
    key = (core * CHUNKS + chunk) * 2 + is_hi
    order = np.argsort(key, kind="stable")
    key_s = key[order]
    dst_s = dst[order]
    slot_s = slot[order]

    nseg = N_CORES * CHUNKS * 2
    counts = np.bincount(key_s, minlength=nseg).reshape(N_CORES, CHUNKS, 2)
    seg_end = np.cumsum(counts.reshape(-1))
    seg_start = seg_end - counts.reshape(-1)

    lo_max = counts[:, :, 0].max(axis=0)  # [CHUNKS]
    hi_max = counts[:, :, 1].max(axis=0)
    lo_blk = np.maximum(1, (lo_max + 127) // 128).astype(np.int64)
    hi_blk = np.maximum(1, (hi_max + 127) // 128).astype(np.int64)

    # common (replicated) tensors
    xs_t = np.zeros((NT_PAD, F), dtype=BF16)
    xs_t[:N_NODES] = (
        np.asarray(x, dtype=np.float32) * dinv[:, None]
    ).astype(BF16)
    wT = np.ascontiguousarray(np.asarray(W, dtype=np.float32).T).astype(BF16)
    wt_in = np.stack([wT[:128], wT[128:]])  # [2,128,F]
    biasw = np.zeros((128, F), dtype=BF16)
    biasw[0, :] = np.asarray(b, dtype=np.float32).astype(BF16)
    iota_t = np.tile(np.arange(128, dtype=np.float32)[None, :], (128, 1)).astype(BF16)
    ident_t = np.eye(128, dtype=np.float32).astype(BF16)
    onehot0 = np.zeros((128, 128), dtype=BF16)
    onehot0[0, :] = 1.0
    common = dict(
        xs=xs_t, wt=wt_in, biasw=biasw, iota=iota_t, ident=ident_t,
        onehot0=onehot0,
    )

    dinv_pad = np.ones(NT_PAD, dtype=np.float32)
    dinv_pad[:N_NODES] = dinv

    sum_lo = int(lo_blk.sum())
    sum_hi = int(hi_blk.sum())
    per_core = []
    for c in range(N_CORES):
        ilo = np.zeros((128, 8 * sum_lo), dtype=np.int16)
        ihi = np.zeros((128, 8 * sum_hi), dtype=np.int16)
        slt = np.zeros((128, sum_lo + sum_hi), dtype=BF16)
        lo_off = hi_off = nb_off = 0
        for k in range(CHUNKS):
            LO, HI = int(lo_blk[k]), int(hi_blk[k])
            s = (c * CHUNKS + k) * 2
            a0, a1 = seg_start[s], seg_end[s]
            b0, b1 = seg_start[s + 1], seg_end[s + 1]
            ilo[:, 8 * lo_off : 8 * (lo_off + LO)] = _pack_idx(dst_s[a0:a1], LO)
            ihi[:, 8 * hi_off : 8 * (hi_off + HI)] = _pack_idx(
                dst_s[b0:b1] - SPLIT, HI
            )
            slt[:, nb_off : nb_off + LO] = _pack_slots(slot_s[a0:a1], LO)
            slt[:, nb_off + LO : nb_off + LO + HI] = _pack_slots(slot_s[b0:b1], HI)
            lo_off += LO
            hi_off += HI
            nb_off += LO + HI
        nchk = np.arange(128)[:, None] + 128 * np.arange(CHUNKS)[None, :] + c * NPC
        dvc = np.where(
            nchk - c * NPC < NPC, dinv_pad[np.minimum(nchk, N_NODES - 1)], 1.0
        ).astype(np.float32)
        per_core.append(
            dict(
                idx_lo=ilo,
                idx_hi=ihi,
                slots=slt,
                dinv_chk=np.ascontiguousarray(dvc),
                xself=np.ascontiguousarray(
                    xs_t[c * NPC : c * NPC + CHUNKS * 128]
                ),
            )
        )
    return lo_blk, hi_blk, common, per_core


def _install_ntff_hook():
    """The agent image's antenv lacks axon_hooks; recreate it so
    run_bass_kernel_spmd(trace=True) can profile via the axon .so."""
    import types

    if "antenv.axon_hooks" in sys.modules:
        return
    mod = types.ModuleType("antenv.axon_hooks")
    state = {}
    mod.set_axon_ntff_profile_hook = lambda h: state.__setitem__("h", h)
    mod.get_axon_ntff_profile_hook = lambda: state.get("h")
    sys.modules["antenv.axon_hooks"] = mod
    try:
        import antenv

        antenv.axon_hooks = mod
    except Exception:
        pass
    try:
        if "/root/.axon_site" not in sys.path:
            sys.path.insert(0, "/root/.axon_site")
        from trn_agent_boot.trn_boot import _ntff_profile_via_ctypes

        mod.set_axon_ntff_profile_hook(
            _ntff_profile_via_ctypes("/opt/axon/libaxon_pjrt.so")
        )
    except Exception:
        pass


_CACHE = {}


def kernel(x, edge_index, W, b, trace=False):
    if trace:
        _install_ntff_hook()
    lo_blk, hi_blk, common, per_core = _prep(x, edge_index, W, b)
    key = (tuple(lo_blk), tuple(hi_blk))
    if key not in _CACHE:
        _CACHE[key] = _build_program(lo_blk, hi_blk)
    nc = _CACHE[key]

    in_maps = []
    for c in range(N_CORES):
        m = dict(common)
        m.update(per_core[c])
        in_maps.append(m)

    res = run_bass_kernel_spmd(
        nc, in_maps, core_ids=list(range(N_CORES)), trace=trace
    )
    out = np.concatenate([r["out"] for r in res.results], axis=0)
    if trace:
        kernel.last_exec_ns = res.exec_time_ns
        kernel.last_profile = res.profile_json
    return out.astype(np.float32)
